# revision 38
# baseline (speedup 1.0000x reference)
"""Trainium2 Bass kernel for nn_Attention_12463995093474 (sparse_attention).

Math (reference):
  q/k/v = content linears; 2 absolute heads, 2 relative heads (DK=32).
  abs:  scores = (Xq_a + abs_kernel@abs_q_w) @ (Xk_a + abs_kernel@abs_k_w)^T
  rel:  scores = Xq_r @ Xk_r^T + (Xq_r + rel_bias) . (rel_kernel@rel_k_w + rel_k_b)
  softmax(mask) @ v -> out linear.

The dominant term is the streaming contraction over rel_kernel (655 MB):
    s2[i,j] = sum_o q'[i,o] * (sum_d RK[i,j,d] rel_k_w[d,o]) (+ const c[i])
This version runs it almost entirely on the TensorEngine:
  1. dma_start_transpose streams RK[i, j, :] chunks HBM->SBUF as [d=128, (i,j)]
     (bf16 xbar transpose).
  2. Stage B (PE): for groups of 4 i's, 4 col-tiled matmuls with
     lhsT = rel_k_w [128d, 32o] write R^T packed as [4i x 32o, 200j] in PSUM.
  3. A PSUM->SBUF bf16 copy (DVE/ACT alternating).
  4. Stage C (PE): per 32-i band, 8 accumulating matmuls with a
     zero-interleaved block-diagonal lhsT holding q' values contract o away,
     producing s2[i-band, j] directly in PSUM (32-aligned output bands).
  5. The content-score matmul for rel heads accumulates into the same PSUM
     tile (start=False), so the epilogue reads finished scores.
This removes the 41M-element DVE multiply+tree-reduce of the previous
version; the kernel becomes transpose-DMA bound instead of DVE bound.

Sharding: data-parallel over batch, B=16 -> 2 batches per core on 8 cores.
"""

import numpy as np
from contextlib import ExitStack

import concourse.bass as bass
import concourse.bacc as bacc
import concourse.tile as tile
from concourse import mybir
from concourse.masks import make_identity
from concourse.bass_utils import run_bass_kernel_spmd

B, T, D = 16, 200, 128
H_ABS, H_REL, H, DK = 2, 2, 4, 32
N_CORES = 8
BPC = B // N_CORES  # batches per core
SCALE = 1.0 / float(DK) ** 0.5
TT = BPC * T  # tokens per core (400)

F32 = mybir.dt.float32
BF16 = mybir.dt.bfloat16
I32 = mybir.dt.int32
AX = mybir.AxisListType
OP = mybir.AluOpType
AF = mybir.ActivationFunctionType

# i-blocks per batch: (start, len)
IBLOCKS = [(0, 128), (128, T - 128)]
DEBUG = False

# all small weights packed host-side into one [128, N] f32 tensor -> 1 DMA
WPACK_LAYOUT = [
    ("Wq", 128, 128), ("Wk", 128, 128), ("Wv", 128, 128), ("Wo", 128, 128),
    ("bq", 128, 1), ("bk", 128, 1),
    ("rkw0", 128, 32), ("rkw1", 128, 32),
    ("rkb0", 32, 1), ("rkb1", 32, 1), ("rbias0", 32, 1), ("rbias1", 32, 1),
    ("aqw0", 128, 32), ("aqw1", 128, 32), ("akw0", 128, 32), ("akw1", 128, 32),
    ("aqb0", 32, 1), ("aqb1", 32, 1), ("akb0", 32, 1), ("akb1", 32, 1),
    ("bvb", 128, 128), ("bob", 128, 128),
]
WPACK_OFF = {}
_o = 0
for _nm, _r, _c in WPACK_LAYOUT:
    WPACK_OFF[_nm] = _o
    _o += _c
WPACK_COLS = _o
WPACK_ROWS = (_o + 15) // 16 * 16
# one host-packed [IN_ROWS, 128] bf16 tensor: weights + q/k/v + absk + masks,
# loaded with a single xbar transpose-DMA
IN_OFF_W = 0
IN_OFF_Q = WPACK_ROWS
IN_OFF_K = IN_OFF_Q + 400
IN_OFF_V = IN_OFF_K + 400
IN_OFF_A0 = IN_OFF_V + 400
IN_OFF_A1 = IN_OFF_A0 + 400
IN_OFF_M = IN_OFF_A1 + 400
IN_ROWS = IN_OFF_M + 4 * 208


def chunks_for(il):
    """i-chunks (local_start, G) for the transpose-DMA stream."""
    out, i = [], 0
    while i < il:
        g = min(32, il - i)
        out.append((i, g))
        i += g
    return out


def build_kernel(ctx: ExitStack, tc: tile.TileContext, io: dict):
    nc = tc.nc

    relk = io["rel_kernel"]    # [2, 2, 200, 200, 128] bf16
    out = io["out"]            # [2, 200, 128]

    consts = ctx.enter_context(tc.tile_pool(name="consts", bufs=1))
    prep = ctx.enter_context(tc.tile_pool(name="prep", bufs=2))
    keep = ctx.enter_context(tc.tile_pool(name="keep", bufs=1))

    # Every input load is a transpose-DMA on the sync ring: Tile serializes
    # DMA-transposes against any concurrent plain DMA (xbar-deadlock guard),
    # so a single all-transpose FIFO stream is the only gap-free shape.

    ident = consts.tile([128, 128], F32, tag="ident")
    make_identity(nc, ident)

    inp = consts.tile([128, IN_ROWS], BF16, tag="inpack")
    nc.sync.dma_start_transpose(inp, io["inpack"])
    wtile = inp[:, :WPACK_ROWS]

    def wslice(nm, rows=128):
        o = WPACK_OFF[nm]
        c = dict((n, cc) for n, _r, cc in WPACK_LAYOUT)[nm]
        return wtile[:rows, o : o + c]

    rkt_pool = ctx.enter_context(tc.tile_pool(name="rkt", bufs=8))
    p4sb_pool = ctx.enter_context(tc.tile_pool(name="p4sb", bufs=66))
    sm = ctx.enter_context(tc.tile_pool(name="sm", bufs=2))
    ps_p4 = ctx.enter_context(tc.tile_pool(name="ps_p4", bufs=2, space="PSUM"))
    ps_s2 = ctx.enter_context(tc.tile_pool(name="ps_s2", bufs=1, space="PSUM"))
    ps_tp = ctx.enter_context(tc.tile_pool(name="ps_tp", bufs=1, space="PSUM"))
    ps_s1 = ctx.enter_context(tc.tile_pool(name="ps_s1", bufs=1, space="PSUM"))
    ps_x = ctx.enter_context(tc.tile_pool(name="ps_x", bufs=1, space="PSUM"))
    ps_y = ctx.enter_context(tc.tile_pool(name="ps_y", bufs=1, space="PSUM"))

    lhsT3 = {}
    c_sb = {}
    _cnt = [0]
    stream_state = {}
    if True:

        # ---- stream prerequisites first ----
        wq = wslice("Wq")
        bq_s = consts.tile([128, 1], F32, tag="bq_s")
        nc.scalar.activation(bq_s, wslice("bq"), AF.Copy, scale=SCALE)

        rkw16 = {}
        small_cols = {}
        for hr in range(H_REL):
            rkw16[hr] = wslice(f"rkw{hr}")
            t32 = consts.tile([DK, 1], F32, tag=f"rkb32_{hr}")
            nc.scalar.copy(t32, wslice(f"rkb{hr}", rows=DK))
            small_cols[("rkb", hr)] = t32
            ts_ = consts.tile([DK, 1], F32, tag=f"rbias_s{hr}")
            nc.scalar.activation(ts_, wslice(f"rbias{hr}", rows=DK), AF.Copy,
                                 scale=SCALE)
            small_cols[("rbias_s", hr)] = ts_

        # Pre-create lhsT3 tiles (filled later in prep); their C-matmuls
        # are emitted in the main loop, after the fill, so deps stay valid.
        for hr in range(H_REL):
            for b in range(BPC):
                for ib in range(2):
                    lhsT3[(hr, b, ib)] = keep.tile(
                        [128, 1024], BF16, tag=f"lt{hr}_{b}_{ib}",
                        name=f"lt{hr}_{b}_{ib}")

        def emit_stream(b, ib):
            """Emit one block's transpose-DMAs + stage-B matmuls + copies.
            Stage-C matmuls are deferred (pending_c) so the PE stream has no
            sem-wait bubbles; for block (0,0) the whole block defers (its
            lhsT3 is filled by prep, which runs concurrently), while later
            blocks flush one chunk behind to shorten the final tail."""
            i0, il = IBLOCKS[ib]
            st = {"s2ps": {}, "pending_c": []}
            defer_all = (b == 0 and ib == 0)
            for hr in range(H_REL):
                s2t = ps_s2.tile([128, T], F32, tag=f"s2h{hr}",
                                 name=f"s2h{hr}")
                st["s2ps"][hr] = s2t
                rkflat = relk[hr, b].flatten_outer_dims()  # [40000, 128]
                for ci, (ic0, G) in enumerate(chunks_for(il)):
                    rows = G * T
                    rkt = rkt_pool.tile([128, 6400], BF16, tag="rkt",
                                        name="rkt")
                    # single HWDGE ring for transposes: alternating rings
                    # corrupts reads (cross-ring completion ordering)
                    nc.sync.dma_start_transpose(
                        rkt[:, :rows],
                        rkflat[(i0 + ic0) * T : (i0 + ic0 + G) * T, :])
                    if not defer_all:
                        # flush previous chunk's stage-C (copies long done)
                        for (o_, l_, r_, st_, sp_, tp_) in st["pending_c"]:
                            nc.tensor.matmul(o_, l_, r_, start=st_, stop=sp_,
                                             skip_group_check=True,
                                             tile_position=tp_)
                        st["pending_c"].clear()
                    for gl in range(G // 4):
                        gi = (ic0 + gl * 4) // 4   # group idx within block
                        p4 = ps_p4.tile([128, T], F32, tag="p4", name="p4")
                        for g in range(4):
                            nc.tensor.matmul(
                                p4[32 * g : 32 * (g + 1), :],
                                rkw16[hr],
                                rkt[:, (gl * 4 + g) * T : (gl * 4 + g + 1) * T],
                                skip_group_check=True,
                                tile_position=(0, 96) if g == 3 else None)
                        p4c = p4sb_pool.tile([128, T], BF16, tag="p4sb",
                                             name="p4sb")
                        _cnt[0] += 1
                        if _cnt[0] % 2 == 0:
                            nc.vector.tensor_copy(p4c, p4)
                        else:
                            nc.scalar.copy(p4c, p4)
                        if DEBUG and hr == 0 and b == 0 and ib == 0:
                            nc.gpsimd.dma_start(io["dbg_p4"][gi], p4c)
                        bnd, k = gi // 8, gi % 8
                        Mb = min(32, il - 32 * bnd)
                        klast = (Mb + 3) // 4 - 1
                        S = bnd * 256 + k * 32
                        st["pending_c"].append((
                            s2t[32 * bnd : 32 * bnd + Mb, :],
                            lhsT3[(hr, b, ib)][:, S : S + Mb],
                            p4c, k == 0, k == klast,
                            (0, 96) if bnd == 3 else None))
            stream_state[(b, ib)] = st

        # hoist block (0,0)'s whole B phase ahead of the rest of prep: its
        # DMAs + stage-B matmuls only need rkw16, so the ring and PE start
        # immediately instead of stalling behind the prep dependency chain
        # everything arrives in the single inpack transpose; just slice
        xqT = inp[:, IN_OFF_Q : IN_OFF_Q + 400]
        xkT_pre = inp[:, IN_OFF_K : IN_OFF_K + 400]
        xvT_pre = inp[:, IN_OFF_V : IN_OFF_V + 400]
        akT_pre = {0: inp[:, IN_OFF_A0 : IN_OFF_A0 + 400],
                   1: inp[:, IN_OFF_A1 : IN_OFF_A1 + 400]}
        mtile_pre = {}
        for b in range(BPC):
            for ib in range(2):
                o = IN_OFF_M + (b * 2 + ib) * 208
                mtile_pre[(b, ib)] = inp[:, o : o + 208]

        emit_stream(0, 0)

        qT = {}
        for h in (H_ABS, H_ABS + 1, 0, 1):  # rel heads first
            qp = ps_p4.tile([DK, TT], F32, tag="p4", name="p4")
            nc.tensor.matmul(qp, wq[:, DK * h : DK * (h + 1)], xqT,
                             skip_group_check=True)
            t = keep.tile([DK, TT], F32, tag=f"qT{h}", name=f"qT{h}")
            nc.scalar.activation(t, qp, AF.Identity,
                                 bias=bq_s[DK * h : DK * (h + 1)], scale=SCALE)
            qT[h] = t

        qrbT = {}
        for hr in range(H_REL):
            t = keep.tile([DK, TT], F32, tag=f"qrbT{hr}", name=f"qrbT{hr}")
            nc.vector.tensor_scalar(t, qT[H_ABS + hr],
                                    small_cols[("rbias_s", hr)], None, OP.add)
            qrbT[hr] = t

        # blockmask[p, g] = 1 if p//32 == g
        bmask = consts.tile([128, 4], F32, tag="bmask")
        nc.vector.memset(bmask, 0.0)
        for g in range(4):
            nc.vector.memset(bmask[32 * g : 32 * (g + 1), g : g + 1], 1.0)

        # per-(hr, b, ib): c_sb (row constant) and the sparse stage-C lhsT
        for hr in range(H_REL):
            for b in range(BPC):
                for ib, (i0, il) in enumerate(IBLOCKS):
                    tsl = slice(b * T + i0, b * T + i0 + il)
                    cp = ps_s1.tile([128, 1], F32, tag="s1", name="s1")
                    nc.tensor.matmul(cp[:il, :], qrbT[hr][:, tsl],
                                     small_cols[("rkb", hr)],
                                     skip_group_check=True)
                    t = keep.tile([128, 1], F32, tag=f"c{hr}_{b}_{ib}",
                                  name=f"c{hr}_{b}_{ib}")
                    nc.scalar.copy(t[:il, :], cp[:il, :])
                    c_sb[(hr, b, ib)] = t

                    n_gi = il // 4
                    # q'pack[32g+o, gi] = qrb[o, t(gi*4+g)] via 4 PE matmuls
                    qp_ps = ps_s1.tile([128, 32], F32, tag="s1",
                                       name="s1")
                    for g in range(4):
                        s0 = b * T + i0 + g
                        src = qrbT[hr][:, s0 : s0 + 4 * (n_gi - 1) + 1 : 4]
                        nc.tensor.matmul(
                            qp_ps[32 * g : 32 * (g + 1), :n_gi],
                            ident[:DK, :DK], src, skip_group_check=True,
                            tile_position=(0, 96) if g == 3 else None)
                    qpk = prep.tile([128, 32], F32, tag="qpk", name="qpk")
                    nc.vector.tensor_copy(qpk[:, :n_gi], qp_ps[:, :n_gi])

                    # zero-interleaved block-diag lhsT: group gi=(bnd,k)'s
                    # 4 diag columns live at col bnd*256 + k*36 (+g); the
                    # matmul window for (bnd,k) is [bnd*256+k*32, +Mb)
                    lt = lhsT3[(hr, b, ib)]
                    nc.vector.memset(lt, 0.0)
                    nb = il // 32          # full 32-i bands
                    rem = (il - 32 * nb) // 4   # leftover 4-i groups
                    rs_l = lt.ap[0][0]
                    rs_q = qpk.ap[0][0]
                    rs_m = bmask.ap[0][0]
                    if nb:
                        o4 = bass.AP(tensor=lt.tensor, offset=lt.offset,
                                     ap=[[rs_l, 128], [256, nb], [36, 8], [1, 4]])
                        i4 = bass.AP(tensor=qpk.tensor, offset=qpk.offset,
                                     ap=[[rs_q, 128], [8, nb], [1, 8], [0, 4]])
                        m4 = bass.AP(tensor=bmask.tensor, offset=bmask.offset,
                                     ap=[[rs_m, 128], [0, nb], [0, 8], [1, 4]])
                        nc.vector.tensor_tensor(o4, i4, m4, op=OP.mult)
                    if rem:
                        o4 = bass.AP(tensor=lt.tensor,
                                     offset=lt.offset + nb * 256,
                                     ap=[[rs_l, 128], [36, rem], [1, 4]])
                        i4 = bass.AP(tensor=qpk.tensor,
                                     offset=qpk.offset + nb * 8,
                                     ap=[[rs_q, 128], [1, rem], [0, 4]])
                        m4 = bass.AP(tensor=bmask.tensor, offset=bmask.offset,
                                     ap=[[rs_m, 128], [0, rem], [1, 4]])
                        nc.vector.tensor_tensor(o4, i4, m4, op=OP.mult)

        # ---- rest of prep ----
        xkT = transpose_in(key, "xkT")
        xvT = transpose_in(value, "xvT")

        wk = load_const("wk", io["Wk"], [128, 128])
        wv = load_const("wv", io["Wv"], [128, 128])
        wo = load_const("wo", io["Wo"], [128, 128])
        bk_c = load_const("bk", io["bk"], [128, 1])
        bv_b = consts.tile([128, 128], F32, tag="bv_b")
        bv_ap = io["bv"]
        small_dma(bv_b, bass.AP(tensor=bv_ap.tensor, offset=bv_ap.offset,
                                ap=[[0, 128]] + bv_ap.ap))
        bo_b = consts.tile([128, 128], F32, tag="bo_b")
        bo_ap = io["bo"]
        small_dma(bo_b, bass.AP(tensor=bo_ap.tensor, offset=bo_ap.offset,
                                ap=[[0, 128]] + bo_ap.ap))

        abs_w = {}
        for hh in range(H_ABS):
            abs_w[("aqw", hh)] = load_const(f"aqw{hh}", io["abs_q_w"][hh], [128, DK])
            abs_w[("akw", hh)] = load_const(f"akw{hh}", io["abs_k_w"][hh], [128, DK])
            small_cols[("akb", hh)] = load_const(
                f"akb{hh}", io["abs_k_b"][hh], [DK, 1])
            t = load_const(f"aqb{hh}", io["abs_q_b"][hh], [DK, 1])
            ts_ = consts.tile([DK, 1], F32, tag=f"aqb_s{hh}")
            nc.scalar.activation(ts_, t, AF.Copy, scale=SCALE)
            small_cols[("aqb_s", hh)] = ts_

        kT = {}
        for h in range(H):
            kp = ps_p4.tile([DK, TT], F32, tag="p4", name="p4")
            nc.tensor.matmul(kp, wk[:, DK * h : DK * (h + 1)], xkT,
                             skip_group_check=True)
            t = keep.tile([DK, TT], F32, tag=f"kT{h}", name=f"kT{h}")
            nc.scalar.activation(t, kp, AF.Identity,
                                 bias=bk_c[DK * h : DK * (h + 1)])
            kT[h] = t

        vb = {}
        for b in range(BPC):
            for jb, (j0, jl) in enumerate(IBLOCKS):
                vp = ps_s1.tile([128, 128], F32, tag="s1", name="s1")
                nc.tensor.matmul(vp[:jl, :], xvT[:, b * T + j0 : b * T + j0 + jl], wv,
                                 skip_group_check=True)
                t = keep.tile([128, 128], F32, tag=f"v{b}_{jb}", name=f"v{b}_{jb}")
                nc.vector.tensor_add(t[:jl, :], vp[:jl, :], bv_b[:jl, :])
                vb[(b, jb)] = t

        qaT = {}
        kaT = {}
        for hh in range(H_ABS):
            akT = transpose_in(absk[hh].flatten_outer_dims(), f"akT{hh}")
            pp = ps_p4.tile([DK, TT], F32, tag="p4", name="p4")
            nc.tensor.matmul(pp, abs_w[("aqw", hh)], akT, skip_group_check=True)
            pqT = prep.tile([DK, TT], F32, tag="pqT", name="pqT")
            nc.scalar.activation(pqT, pp, AF.Identity,
                                 bias=small_cols[("aqb_s", hh)], scale=SCALE)
            t = keep.tile([DK, TT], F32, tag=f"qaT{hh}", name=f"qaT{hh}")
            nc.vector.tensor_add(t, qT[hh], pqT)
            qaT[hh] = t

            pp2 = ps_p4.tile([DK, TT], F32, tag="p4", name="p4")
            nc.tensor.matmul(pp2, abs_w[("akw", hh)], akT, skip_group_check=True)
            pkT = prep.tile([DK, TT], F32, tag="pqT", name="pqT")
            nc.scalar.activation(pkT, pp2, AF.Identity,
                                 bias=small_cols[("akb", hh)])
            t = keep.tile([DK, TT], F32, tag=f"kaT{hh}", name=f"kaT{hh}")
            nc.vector.tensor_add(t, kT[hh], pkT)
            kaT[hh] = t

        mb = {}
        for b in range(BPC):
            for ib, (i0, il) in enumerate(IBLOCKS):
                mi = prep.tile([128, T], I32, tag="m_i32", name="m_i32")
                small_dma(mi[:il, :], mask[b, 0, i0 : i0 + il, :])
                t = keep.tile([128, T], F32, tag=f"mb{b}_{ib}", name=f"mb{b}_{ib}")
                nc.vector.tensor_scalar(t[:il, :], mi[:il, :], 1e9, -1e9,
                                        OP.mult, OP.add)
                mb[(b, ib)] = t

    # ---------------- main phase ----------------

    out_stores = []
    for b in range(BPC):
        for ib, (i0, il) in enumerate(IBLOCKS):
            tsl = slice(b * T + i0, b * T + i0 + il)
            bsl = slice(b * T, (b + 1) * T)
            last_blk = (b == BPC - 1 and ib == 1)
            xT_ps = ps_x.tile([128, 128], F32, tag="xT", name="xT")

            def epi_head(h, s2ps):
                is_rel = h >= H_ABS
                st = sm.tile([128, T], F32, tag="st", name="st")
                if is_rel:
                    hr = h - H_ABS
                    nc.tensor.matmul(s2ps[hr][:il, :], qT[h][:, tsl],
                                     kT[h][:, bsl], start=False, stop=True,
                                     skip_group_check=True)
                    nc.vector.tensor_scalar(st[:il, :], s2ps[hr][:il, :],
                                            c_sb[(hr, b, ib)][:il], None,
                                            OP.add)
                    nc.vector.tensor_add(st[:il, :], st[:il, :],
                                         mb[(b, ib)][:il, :])
                else:
                    s1 = ps_s1.tile([128, T], F32, tag="s1", name="s1")
                    nc.tensor.matmul(s1[:il, :], qaT[h][:, tsl],
                                     kaT[h][:, bsl], skip_group_check=True)
                    nc.vector.tensor_add(st[:il, :], s1[:il, :],
                                         mb[(b, ib)][:il, :])

                nmax = sm.tile([128, 1], F32, tag="nmax", name="nmax")
                nc.vector.tensor_reduce(nmax[:il], st[:il, :], AX.X, OP.max,
                                        negate=True)
                p = sm.tile([128, T], F32, tag="p", name="p")
                rsum = sm.tile([128, 1], F32, tag="rsum", name="rsum")
                nc.scalar.activation(p[:il, :], st[:il, :], AF.Exp,
                                     bias=nmax[:il], accum_out=rsum[:il])
                rcp = sm.tile([128, 1], F32, tag="rcp", name="rcp")
                nc.vector.reciprocal(rcp[:il], rsum[:il])
                nc.vector.tensor_scalar(p[:il, :], p[:il, :], rcp[:il], None,
                                        OP.mult)
                if DEBUG:
                    nc.gpsimd.dma_start(
                        io["dbg_st"][b, ib, h, :il, :], st[:il, :])

                hsl = slice(DK * h, DK * (h + 1))
                for jb, (j0, jl) in enumerate(IBLOCKS):
                    tp = ps_tp.tile([128, 128], F32, tag="tp", name="tp")
                    nc.tensor.matmul(tp[:jl, :il], p[:il, j0 : j0 + jl],
                                     ident[:il, :il], is_transpose=True,
                                     skip_group_check=True)
                    pT = sm.tile([128, 128], F32, tag="pT", name="pT")
                    nc.scalar.copy(pT[:jl, :il], tp[:jl, :il])
                    nc.tensor.matmul(xT_ps[hsl, :il], vb[(b, jb)][:jl, hsl],
                                     pT[:jl, :il],
                                     start=(jb == 0), stop=(jb == 1),
                                     skip_group_check=True,
                                     tile_position=(0, 96) if h == 3 else None)

            if last_blk:
                # abs heads don't need the rel stream: emit them BEFORE the
                # last block's stage-B so they run during streaming instead
                # of in the serial tail
                epi_head(0, None)
                epi_head(1, None)
            if (b, ib) not in stream_state:
                emit_stream(b, ib)
            sstate = stream_state[(b, ib)]
            s2ps = sstate["s2ps"]
            for (ap_out, ap_l, ap_r, st_, sp_, tp_) in sstate["pending_c"]:
                nc.tensor.matmul(ap_out, ap_l, ap_r, start=st_, stop=sp_,
                                 skip_group_check=True, tile_position=tp_)
            sstate["pending_c"].clear()

            if DEBUG and b == 0 and ib == 0:
                for hr_ in range(H_REL):
                    s2dump = sm.tile([128, T], F32, tag="s2dump",
                                     name="s2dump")
                    nc.vector.tensor_copy(s2dump[:il, :], s2ps[hr_][:il, :])
                    nc.gpsimd.dma_start(io["dbg_s2"][hr_, :il, :],
                                        s2dump[:il, :])

            # ---- epilogue: scores -> softmax -> p@v -> out linear ----
            for h in ((2, 3) if last_blk else (2, 3, 0, 1)):
                epi_head(h, s2ps)

            xT_sb = sm.tile([128, 128], F32, tag="xT_sb", name="xT_sb")
            nc.scalar.copy(xT_sb[:, :il], xT_ps[:, :il])
            y_ps = ps_y.tile([128, 128], F32, tag="y", name="y")
            nc.tensor.matmul(y_ps[:il, :], xT_sb[:, :il], wo,
                             skip_group_check=True)
            y_sb = sm.tile([128, 128], F32, tag="y_sb", name="y_sb")
            nc.vector.tensor_add(y_sb[:il, :], y_ps[:il, :], bo_b[:il, :])
            nc.gpsimd.dma_start(out[b, i0 : i0 + il, :], y_sb[:il, :])


def build_nc():
    nc = bacc.Bacc(trn_type="TRN2")
    io = {}
    io["inpack"] = nc.dram_tensor(
        "inpack", [IN_ROWS, 128], BF16, kind="ExternalInput").ap()
    io["rel_kernel"] = nc.dram_tensor(
        "rel_kernel", [H_REL, BPC, T, T, D], BF16, kind="ExternalInput"
    ).ap()
    io["out"] = nc.dram_tensor("out", [BPC, T, D], F32, kind="ExternalOutput").ap()
    if DEBUG:
        io["dbg_st"] = nc.dram_tensor(
            "dbg_st", [BPC, 2, H, 128, T], F32, kind="ExternalOutput").ap()
        io["dbg_p4"] = nc.dram_tensor(
            "dbg_p4", [32, 128, T], BF16, kind="ExternalOutput").ap()
        io["dbg_s2"] = nc.dram_tensor(
            "dbg_s2", [H_REL, 128, T], F32, kind="ExternalOutput").ap()

    with tile.TileContext(nc) as tc:
        with ExitStack() as ctx:
            build_kernel(ctx, tc, io)
    nc.compile()
    return nc


_NC_CACHE = None


def _get_nc():
    global _NC_CACHE
    if _NC_CACHE is None:
        _NC_CACHE = build_nc()
    return _NC_CACHE


def make_in_maps(inputs):
    """Shard full inputs into per-core input maps."""
    f32 = np.float32
    g = {k: np.asarray(inputs[k], dtype=f32) for k in
         ["Wq", "bq", "Wk", "bk", "Wv", "bv", "abs_q_w", "abs_q_b",
          "abs_k_w", "abs_k_b", "rel_k_w", "rel_k_b", "rel_bias",
          "Wo", "bo"]}
    wp = np.zeros((128, WPACK_COLS), f32)

    def put(nm, arr):
        o = WPACK_OFF[nm]
        arr = np.asarray(arr, f32)
        if arr.ndim == 1:
            arr = arr[:, None]
        wp[: arr.shape[0], o : o + arr.shape[1]] = arr

    put("Wq", g["Wq"]); put("Wk", g["Wk"]); put("Wv", g["Wv"])
    put("Wo", g["Wo"]); put("bq", g["bq"]); put("bk", g["bk"])
    for hr in range(H_REL):
        put(f"rkw{hr}", g["rel_k_w"][hr])
        put(f"rkb{hr}", g["rel_k_b"][hr])
        put(f"rbias{hr}", g["rel_bias"][0, hr, 0, :])
    for hh in range(H_ABS):
        put(f"aqw{hh}", g["abs_q_w"][hh])
        put(f"akw{hh}", g["abs_k_w"][hh])
        put(f"aqb{hh}", g["abs_q_b"][hh])
        put(f"akb{hh}", g["abs_k_b"][hh])
    put("bvb", np.tile(g["bv"][None, :], (128, 1)))
    put("bob", np.tile(g["bo"][None, :], (128, 1)))
    import ml_dtypes
    bf = ml_dtypes.bfloat16
    # wpack stored transposed (host) so an xbar transpose-DMA yields [128, N]
    wpT = np.zeros((WPACK_ROWS, 128), np.float32)
    wpT[:WPACK_COLS, :] = wp.T
    weights = {}
    query = np.asarray(inputs["query"], dtype=f32).astype(bf)
    key = np.asarray(inputs["key"], dtype=f32).astype(bf)
    value = np.asarray(inputs["value"], dtype=f32).astype(bf)
    # mask pre-transposed+padded on host: [B, 2, 208 (j pad), 128 (i)] bf16
    mask_i = np.asarray(inputs["mask"], dtype=np.int32)[:, 0]  # [B, T, T]
    maskT = np.zeros((B, 2, 208, 128), f32)
    for ib, (i0, il) in enumerate([(0, 128), (128, 72)]):
        maskT[:, ib, :T, :il] = mask_i[:, i0:i0+il, :].transpose(0, 2, 1)
    maskT = maskT.astype(bf)
    relk = np.asarray(inputs["rel_kernel"], dtype=f32).astype(bf)
    absk = np.asarray(inputs["abs_kernel"], dtype=f32).astype(bf)

    in_maps = []
    for c in range(N_CORES):
        bs = slice(c * BPC, (c + 1) * BPC)
        m = dict(weights)
        ip = np.zeros((IN_ROWS, 128), np.float32)
        ip[:WPACK_ROWS] = wpT.astype(np.float32)
        ip[IN_OFF_Q : IN_OFF_Q + 400] = query[bs].reshape(400, 128)
        ip[IN_OFF_K : IN_OFF_K + 400] = key[bs].reshape(400, 128)
        ip[IN_OFF_V : IN_OFF_V + 400] = value[bs].reshape(400, 128)
        ip[IN_OFF_A0 : IN_OFF_A0 + 400] = absk[0, bs].reshape(400, 128)
        ip[IN_OFF_A1 : IN_OFF_A1 + 400] = absk[1, bs].reshape(400, 128)
        ip[IN_OFF_M : IN_OFF_M + 832] = maskT[bs].reshape(832, 128)
        m["inpack"] = np.ascontiguousarray(ip.astype(bf))
        m["rel_kernel"] = np.ascontiguousarray(relk[:, bs])
        in_maps.append(m)
    return in_maps


def kernel(**inputs) -> np.ndarray:
    nc = _get_nc()
    in_maps = make_in_maps(inputs)
    res = run_bass_kernel_spmd(nc, in_maps, core_ids=list(range(N_CORES)))
    return np.concatenate([r["out"] for r in res.results], axis=0)


if __name__ == "__main__":
    nc = build_nc()
    print("built ok")
